# revision 25
# baseline (speedup 1.0000x reference)
"""Bass/Tile TRN2 kernel for a non-local attention block (BaseNonLocalBlock).

Contract: kernel(**inputs) takes the FULL inputs of the nn.Module problem
(B=1, D=256, H=4, N=4096) and returns the FULL output [1, 256, 4096].

Sharding: query columns of the N x N attention are split across the 8
NeuronCores (512 queries per core). K/V projections are computed
redundantly on every core (cheap); each core produces its own output
column slice and the host concatenates.

Per-core structure (flash-attention style, scores never hit HBM):
  pre-phase: Q/K/V conv1x1 projections as fp8 DoubleRow matmuls
    (channel pairs packed planar [128, 2, *]; weights prescaled x16 on
    the host, un-scaled for free in the PSUM->SBUF copy).  K -> bf16
    [256, 4096]; V^T(+ones col per head) -> fp8 [4096, 4*68].
  loop over 32 key chunks (128 keys each):
    S_T = K_h[:, chunk]^T @ Q_h      (PSUM, 2 heads per row-split pair)
    el  = spt * S_T                  (DVE mult, the pace-setting op)
    e2  = exp(el)                    (one fused ACT exp per iter -> fp8)
    msg_h += vt^T @ e2               (fp8 DoubleRow: 1 matmul per head
                                      per TWO chunks, keys double-pumped)
  tail: denominators (row 64 of mps) -> PE ones-broadcast ->
    reciprocal_approx_fast -> per-head mult (co half 0 first so the MLP
    u1 matmul starts early) -> conv MLP with BN folded -> residual add.

The loop is paced by the DVE mask-multiply (PSUM fp32 read, 1 elem/
cycle/lane, ~2.2us per iteration) -- scores and messages fit in the
PE's slack, exp in ACT's.  fp8 projections keep the pre-phase at the
copy roofline instead of the PE's.
"""

import numpy as np
from contextlib import ExitStack

D = 256
N = 4096
NQ = 512          # queries per core
H = 4
DH = 64
NCORES = 8
NIT = N // 128    # 32 key chunks
NPAIR = NIT // 2  # 16 chunk pairs (fp8 DoubleRow message matmuls)
VTS = 68          # padded per-head stride in the V_T-aug tile
WS = 16.0         # host prescale on conv weights before fp8 quantization

_CACHE = {}


def _build(has_bq, has_bk, has_bv, has_b3):
    import concourse.bass as bass
    import concourse.tile as tile
    from concourse import bacc, mybir

    F32 = mybir.dt.float32
    BF16 = mybir.dt.bfloat16
    FP8 = mybir.dt.float8e4
    Id = mybir.ActivationFunctionType.Identity
    Exp = mybir.ActivationFunctionType.Exp
    Relu = mybir.ActivationFunctionType.Relu
    DR = mybir.MatmulPerfMode.DoubleRow

    nc = bacc.Bacc("TRN2", target_bir_lowering=False, debug=False,
                   num_devices=NCORES)

    # DRAM I/O (per core)
    x8_d = nc.dram_tensor("x8", [4, 128, 2, N // 4], FP8,
                          kind="ExternalInput").ap()
    xq8_d = nc.dram_tensor("xq8", [128, 2, NQ], FP8, kind="ExternalInput").ap()
    xqr_d = nc.dram_tensor("xqr", [D, NQ], F32, kind="ExternalInput").ap()
    spt_d = nc.dram_tensor("spt", [N, NQ], BF16, kind="ExternalInput").ap()
    wq8_d = nc.dram_tensor("wq8", [128, 2, D], FP8, kind="ExternalInput").ap()
    wk8_d = nc.dram_tensor("wk8", [128, 2, D], FP8, kind="ExternalInput").ap()
    wv8_d = nc.dram_tensor("wv8", [128, 2, D], FP8, kind="ExternalInput").ap()
    w1t_d = nc.dram_tensor("w1t", [D, 128], BF16, kind="ExternalInput").ap()
    w2t_d = nc.dram_tensor("w2t", [128, 128], BF16, kind="ExternalInput").ap()
    w3t_d = nc.dram_tensor("w3t", [128, D], BF16, kind="ExternalInput").ap()
    bq_d = nc.dram_tensor("bq2", [128, 2], F32, kind="ExternalInput").ap()
    bk_d = nc.dram_tensor("bk2", [128, 2], F32, kind="ExternalInput").ap()
    bv_d = nc.dram_tensor("bv2", [128, 2], F32, kind="ExternalInput").ap()
    b1_d = nc.dram_tensor("b1f", [128, 1], F32, kind="ExternalInput").ap()
    b2_d = nc.dram_tensor("b2f", [128, 1], F32, kind="ExternalInput").ap()
    b3_d = nc.dram_tensor("b32", [128, 2], F32, kind="ExternalInput").ap()
    out_d = nc.dram_tensor("out", [D, NQ], F32, kind="ExternalOutput").ap()

    spt_t3 = spt_d.rearrange("(t p) o -> t p o", p=128)

    with tile.TileContext(nc) as tc, ExitStack() as ctx:
        sb = ctx.enter_context(tc.tile_pool(name="sb", bufs=1))
        spt_pool = ctx.enter_context(tc.tile_pool(name="sptp", bufs=8))
        el_pool = ctx.enter_context(tc.tile_pool(name="elp", bufs=3))
        e2_pool = ctx.enter_context(tc.tile_pool(name="e2p", bufs=3))
        pj_ctx = ExitStack()
        pj = pj_ctx.enter_context(tc.tile_pool(name="pj", bufs=3, space="PSUM"))

        # ---- early inputs: first x quarter + weights, then the rest of x ----
        x8t = [sb.tile([128, 2, 1024], FP8, name=f"x8_{k}") for k in range(4)]
        nc.sync.dma_start(x8t[0][:], x8_d[0])
        xq8 = sb.tile([128, 2, NQ], FP8, name="xq8")
        nc.sync.dma_start(xq8[:], xq8_d[:, :, :])
        wq8 = sb.tile([128, 2, D], FP8, name="wq8")
        wk8 = sb.tile([128, 2, D], FP8, name="wk8")
        wv8 = sb.tile([128, 2, D], FP8, name="wv8")
        nc.sync.dma_start(wk8[:], wk8_d[:, :, :])
        nc.sync.dma_start(wv8[:], wv8_d[:, :, :])
        nc.sync.dma_start(wq8[:], wq8_d[:, :, :])
        if has_bq:
            bq = sb.tile([128, 2], F32, name="bq")
            nc.sync.dma_start(bq[:], bq_d[:, :])
        if has_bk:
            bk = sb.tile([128, 2], F32, name="bk")
            nc.sync.dma_start(bk[:], bk_d[:, :])
        for k in range(1, 4):
            nc.sync.dma_start(x8t[k][:], x8_d[k])

        k_sb = sb.tile([128, 2, N], BF16, name="ksb")
        q_sb = [sb.tile([128, NQ], BF16, name=f"q{co}") for co in range(2)]
        # V^T augmented: per key-chunk it, per head h: [64 V cols | ones | pad]
        vt = sb.tile([128, NIT, H, VTS], FP8, name="vt")
        nc.gpsimd.memset(vt[:, :, :, 64:65], 1.0)
        ones64 = sb.tile([1, 64], BF16, name="ones64")
        nc.gpsimd.memset(ones64[:], 1.0)

        # ---- PE warmup: ~3.5us of tiny matmuls during the DMA ramp so the
        # HAM clock gate is already at 8/8 when real projections start ----
        warm = sb.tile([128, 64], BF16, name="warm")
        nc.vector.memset(warm[:].bitcast(F32)[:, 0:32], 0.0)
        wps = pj.tile([128, NQ], F32, tag="t")
        for r in range(16):
            nc.tensor.matmul(wps[0:64, 0:64], warm[:], warm[:],
                             start=True, stop=True)

        def keep_warm(ap, n):
            # dummy matmuls into a PSUM region that a later start=True matmul
            # fully overwrites; fills PE idle gaps so the HAM clock stays 8/8
            for r in range(n):
                nc.tensor.matmul(ap, warm[:], warm[:], start=True, stop=True)

        # spt prefetch on the (otherwise idle) GPSIMD DMA ring
        spt_tiles = {}

        def load_spt(it):
            t = spt_pool.tile([128, NQ], BF16, tag="spt")
            nc.gpsimd.dma_start(t[:], spt_t3[it])
            spt_tiles[it] = t

        for it in range(6):
            load_spt(it)

        def q_proj():
            # fp8 DoubleRow conv1x1, contraction = 256 channels
            for co in range(2):
                ps = pj.tile([128, NQ], F32, tag="t")
                nc.tensor.matmul(ps[:], wq8[:, :, co * 128:(co + 1) * 128],
                                 xq8[:], start=True, stop=True, perf_mode=DR)
                if has_bq:
                    nc.scalar.activation(q_sb[co][:], ps[:], Id,
                                         scale=1.0 / (WS * 8.0),
                                         bias=bq[:, co:co + 1])
                else:
                    nc.scalar.activation(q_sb[co][:], ps[:], Id,
                                         scale=1.0 / (WS * 8.0))

        # ---- K / V^T projections per 512-key block, copies chase on
        # alternating ACT/DVE ----
        cp = [0]

        def copy_scaled(dst, src, bias=None):
            if bias is not None:
                nc.scalar.activation(dst, src, Id, scale=1.0 / WS, bias=bias)
            elif cp[0] % 2 == 0:
                nc.scalar.activation(dst, src, Id, scale=1.0 / WS)
            else:
                nc.vector.tensor_scalar_mul(dst, src, 1.0 / WS)
            cp[0] += 1

        for ib in range(8):
            xt = x8t[ib // 2]
            xo = (ib % 2) * 512
            kps = pj.tile([128, 2, NQ], F32, tag="t")
            for co in range(2):
                nc.tensor.matmul(kps[:, co, :],
                                 wk8[:, :, co * 128:(co + 1) * 128],
                                 xt[:, :, xo:xo + 512],
                                 start=True, stop=True, perf_mode=DR)
            ksl = k_sb[:, :, ib * 512:(ib + 1) * 512]
            if has_bk:
                for co in range(2):
                    nc.scalar.activation(ksl[:, co, :], kps[:, co, :], Id,
                                         scale=1.0 / WS, bias=bk[:, co:co + 1])
            else:
                copy_scaled(ksl, kps[:])
            vps = pj.tile([128, 4, 256], F32, tag="t")
            for u in range(4):
                ko = xo + u * 128
                nc.tensor.matmul(vps[:, u, 0:D], xt[:, :, ko:ko + 128],
                                 wv8[:], start=True, stop=True,
                                 perf_mode=DR)
            vdst = vt[:, ib * 4:ib * 4 + 4, :, 0:64]
            vsrc = vps[:, 0:4, 0:D].rearrange("p w (h c) -> p w h c", h=H)
            copy_scaled(vdst, vsrc)
            if ib == 0:
                q_proj()

        pj_ctx.close()
        ps_m = ctx.enter_context(tc.tile_pool(name="psm", bufs=1, space="PSUM"))
        pst_ctx = ExitStack()
        ps_t = pst_ctx.enter_context(
            tc.tile_pool(name="pst", bufs=2, space="PSUM"))
        # messages for all 4 heads in one 4-bank PSUM tensor; row 64 of each
        # bank accumulates the softmax denominator (ones column in vt)
        mps = ps_m.tile([65, H, NQ], F32, name="mps")

        # ---- attention loop ----
        pend = None

        def emit_msgs(p):
            pit, e2p = p
            for h in range(H):
                nc.tensor.matmul(mps[:, h, :], vt[:, pit, h, 0:65],
                                 e2p[:, h, :],
                                 start=(pit == 0), stop=(pit == NIT - 1),
                                 skip_group_check=True)

        for it in range(NIT):
            if it + 6 < NIT:
                load_spt(it + 6)
            spt_t = spt_tiles.pop(it)
            # broadcast the mask over the head pair (free-dim 0-stride)
            spt_b = bass.AP(tensor=spt_t.tensor, offset=spt_t.offset,
                            ap=[list(spt_t.ap[0]), [0, 2],
                                list(spt_t.ap[1])])
            e2 = e2_pool.tile([128, H, NQ], FP8, tag="e2")
            el = el_pool.tile([128, H, NQ], BF16, tag="el")
            for hp in range(2):
                sps = ps_t.tile([128, 2, NQ], F32, tag="t")
                if it < 2:
                    keep_warm(sps[0:64, 0, 0:64], 4)
                for jj in range(2):
                    ro = jj * 64
                    nc.tensor.matmul(
                        sps[:, jj, :],
                        k_sb[ro:ro + 64, hp, it * 128:(it + 1) * 128],
                        q_sb[hp][ro:ro + 64, :],
                        start=True, stop=True)
                nc.vector.tensor_mul(el[:, 2 * hp:2 * hp + 2, :], sps[:], spt_b)
            if pend is not None:
                emit_msgs(pend)
                pend = None
            nc.scalar.activation(e2[:], el[:], Exp)
            pend = (it, e2)
        dummy_ps = ps_t.tile([128, 2, NQ], F32, tag="t")
        keep_warm(dummy_ps[0:64, 0, 0:64], 14)
        emit_msgs(pend)

        # ---- late inputs (only needed after the attention loop) ----
        w1t = [sb.tile([128, 128], BF16, name=f"w1t{ci}") for ci in range(2)]
        for ci in range(2):
            nc.sync.dma_start(w1t[ci][:], w1t_d[ci * 128:(ci + 1) * 128, :])
        w2t = sb.tile([128, 128], BF16, name="w2t")
        nc.sync.dma_start(w2t[:], w2t_d[:, :])
        w3t = sb.tile([128, D], BF16, name="w3t")
        nc.sync.dma_start(w3t[:], w3t_d[:, :])
        xqr = [sb.tile([128, NQ], F32, name=f"xqr{co}") for co in range(2)]
        for co in range(2):
            nc.sync.dma_start(xqr[co][:], xqr_d[co * 128:(co + 1) * 128, :])
        b1 = sb.tile([128, 1], F32, name="b1")
        b2 = sb.tile([128, 1], F32, name="b2")
        nc.sync.dma_start(b1[:], b1_d[:, :])
        nc.sync.dma_start(b2[:], b2_d[:, :])
        if has_bv:
            bv = sb.tile([128, 2], F32, name="bv")
            nc.sync.dma_start(bv[:], bv_d[:, :])
        if has_b3:
            b3 = sb.tile([128, 2], F32, name="b3")
            nc.sync.dma_start(b3[:], b3_d[:, :])

        pst_ctx.close()
        pt = ctx.enter_context(tc.tile_pool(name="pt", bufs=2, space="PSUM"))

        # ---- softmax normalization: denominators -> PE broadcast ->
        # fast reciprocal -> per-head multiply (co=0 heads first).
        # DVE handles the critical chain (recip + co=0 mults); the co=1
        # numerators are staged to SBUF by ACT so GPSIMD can multiply them,
        # keeping the DVE free for the residual adds. ----
        dhs = sb.tile([1, H, NQ], BF16, name="dhs")
        nc.scalar.copy(dhs[:, 0:2, :], mps[64:65, 0:2, :])
        nc.vector.tensor_copy(dhs[:, 2:4, :], mps[64:65, 2:4, :])
        mn1 = sb.tile([64, 2, NQ], F32, name="mn1")
        nc.scalar.copy(mn1[:], mps[0:64, 2:4, :])
        msg = [sb.tile([128, NQ], BF16, name=f"msg{co}") for co in range(2)]
        rbc = sb.tile([64, 2, 2, NQ], F32, name="rbc")
        for co in range(2):
            dbb = pt.tile([64, 2, NQ], F32, tag="t")
            keep_warm(dbb[0:64, 0, 0:64], 4)
            for jj in range(2):
                nc.tensor.matmul(dbb[:, jj, :], ones64[:],
                                 dhs[:, 2 * co + jj, :],
                                 start=True, stop=True)
            nc.vector.reciprocal_approx_fast(out=rbc[:, co, :, :], in_=dbb[:])
            for jj in range(2):
                h = 2 * co + jj
                ro = jj * 64
                if co == 0:
                    nc.vector.tensor_mul(msg[co][ro:ro + 64, :],
                                         mps[0:64, h, :], rbc[:, co, jj, :])
                else:
                    nc.gpsimd.tensor_mul(msg[co][ro:ro + 64, :],
                                         mn1[:, jj, :], rbc[:, co, jj, :])
                if has_bv:
                    nc.scalar.activation(msg[co][ro:ro + 64, :],
                                         msg[co][ro:ro + 64, :], Id,
                                         bias=bv[ro:ro + 64, co:co + 1])

        # ---- message MLP + residual ----
        u1 = pt.tile([128, NQ], F32, tag="t")
        keep_warm(u1[0:64, 0:64], 4)
        for ci in range(2):
            nc.tensor.matmul(u1[:], w1t[ci][:], msg[ci][:],
                             start=(ci == 0), stop=(ci == 1))
        h1 = sb.tile([128, NQ], BF16, name="h1")
        nc.scalar.activation(h1[:], u1[:], Relu, bias=b1[:, 0:1])
        u2 = pt.tile([128, NQ], F32, tag="t")
        keep_warm(u2[0:64, 0:64], 3)
        nc.tensor.matmul(u2[:], w2t[:], h1[:], start=True, stop=True)
        h2 = sb.tile([128, NQ], BF16, name="h2")
        nc.scalar.activation(h2[:], u2[:], Relu, bias=b2[:, 0:1])
        for co in range(2):
            u3 = pt.tile([128, NQ], F32, tag="t")
            keep_warm(u3[0:64, 0:64], 3)
            nc.tensor.matmul(u3[:], w3t[:, co * 128:(co + 1) * 128],
                             h2[:], start=True, stop=True)
            ot = sb.tile([128, NQ], F32, name=f"ot{co}")
            if has_b3:
                tb = sb.tile([128, NQ], F32, name=f"tb{co}")
                nc.scalar.activation(tb[:], u3[:], Id, bias=b3[:, co:co + 1])
                nc.vector.tensor_add(ot[:], tb[:], xqr[co][:])
            else:
                nc.vector.tensor_add(ot[:], u3[:], xqr[co][:])
            nc.sync.dma_start(out_d[co * 128:(co + 1) * 128, :], ot[:])

    nc.compile()
    return nc


def _prep_inputs(inputs):
    import ml_dtypes
    E4 = ml_dtypes.float8_e4m3
    bf = lambda a: np.ascontiguousarray(
        np.asarray(a, dtype=np.float32).astype(ml_dtypes.bfloat16))
    f8 = lambda a: np.ascontiguousarray(
        np.asarray(a, dtype=np.float32).astype(E4))
    f = lambda a: np.ascontiguousarray(np.asarray(a, dtype=np.float32))
    planar = lambda a: np.ascontiguousarray(
        np.asarray(a, np.float32).reshape(2, 128, -1).transpose(1, 0, 2))

    x32 = f(inputs["corr_feat_belief"][0])                  # [D, N]
    spT = np.asarray(inputs["spatial_compatibility"][0]).T  # [N(keys), N(q)]
    Wq, bq = f(inputs["Wq"]), f(inputs["bq"])
    Wk, bk = f(inputs["Wk"]), f(inputs["bk"])
    Wv, bv = f(inputs["Wv"]), f(inputs["bv"])
    W1, b1, g1, be1 = f(inputs["W1"]), f(inputs["b1"]), f(inputs["g1"]), f(inputs["be1"])
    W2, b2, g2, be2 = f(inputs["W2"]), f(inputs["b2"]), f(inputs["g2"]), f(inputs["be2"])
    W3, b3 = f(inputs["W3"]), f(inputs["b3"])

    scale = np.float32(1.0 / np.sqrt(DH))
    s1 = (g1 / np.sqrt(np.float32(1.0) + np.float32(1e-5))).astype(np.float32)
    s2 = (g2 / np.sqrt(np.float32(1.0) + np.float32(1e-5))).astype(np.float32)

    xpl = planar(x32)               # [128, 2, N]; channel c = p + 128*j
    x8 = f8(xpl)
    # quarter-major so each quarter DMA reads contiguous 2KB/partition lines
    x8q = np.ascontiguousarray(
        np.stack([x8[:, :, k * 1024:(k + 1) * 1024] for k in range(4)]))
    spT_bf = bf(spT)
    common = dict(
        x8=x8q,
        wq8=f8(planar(Wq.T) * WS),
        wk8=f8(planar(Wk.T) * WS),
        wv8=f8(planar(Wv.T) * WS),
        w1t=bf((W1 * s1[:, None]).T),
        w2t=bf((W2 * s2[:, None]).T),
        w3t=bf(W3.T),
        bq2=f((bq * scale).reshape(2, 128).T),
        bk2=f(bk.reshape(2, 128).T),
        bv2=f(bv.reshape(2, 128).T),
        b1f=f((s1 * b1 + be1).reshape(128, 1)),
        b2f=f((s2 * b2 + be2).reshape(128, 1)),
        b32=f(b3.reshape(2, 128).T),
    )
    in_maps = []
    for m in range(NCORES):
        sl = slice(m * NQ, (m + 1) * NQ)
        im = dict(common)
        im["xq8"] = np.ascontiguousarray(x8[:, :, sl])
        im["xqr"] = f(x32[:, sl])
        im["spt"] = np.ascontiguousarray(spT_bf[:, sl])
        in_maps.append(im)
    flags = tuple(bool(np.any(b != 0)) for b in (bq, bk, bv, b3))
    return in_maps, flags


def _run(inputs, trace=False):
    from concourse.bass_utils import run_bass_kernel_spmd
    in_maps, flags = _prep_inputs(inputs)
    if flags not in _CACHE:
        _CACHE[flags] = _build(*flags)
    nc = _CACHE[flags]
    res = run_bass_kernel_spmd(nc, in_maps, core_ids=list(range(NCORES)),
                               trace=trace)
    out = np.concatenate([res.results[m]["out"] for m in range(NCORES)],
                         axis=1)[None]
    return np.ascontiguousarray(out.astype(np.float32)), res


def kernel(**inputs):
    out, _ = _run(inputs, trace=False)
    return out
